# revision 23
# baseline (speedup 1.0000x reference)
"""Trainium2 Bass kernel for nn_CrossAttentionLayer (B=4, N=1024, M=4096,
DQ=DC=1024, H=16, DH=64).

Sharding: 8 cores = 4 batches x 2 half-head-groups. Core c handles batch
c//2 and heads [8*(c%2), 8*(c%2)+8). Each core computes its partial
out = concat_heads(attn) @ Wo_slice; host sums the two partials per batch
and adds the bias.

Math identity on device:
  P = clamp(exp(scale*S + madd), e^-5, e^5)   with madd = 0 / -1000 (mask)
equals exp(clip(where(mask, scale*S, -inf), -5, 5)).  x^T/ctx^T/Wq/Wkv are
cast + transposed on the host so the kernel does NO PE transposes; K^T,
Q^T and V stay resident in SBUF.

Dtypes: projections and S in bf16 (fp8 there fails the 2e-2 tolerance);
P@V runs in fp8-e4m3 DoubleRow (end-to-end max-rel-err 1.8e-2 emulated):
V is stored fp8 in chunk-pair layout [128, h(8), 2, 96] (64 V dims + 32
ones columns - DoubleRow stationary width must be a multiple of 32; P@V
output rows 64..95 accumulate sum(P), the softmax denominator) and the
exp writes P fp8 into pair tiles [128, 2*1024]; one DoubleRow matmul per
(chunk-pair, head) contracts both 128-row chunks at 0.5 cycles/column.
exp overflow saturates fp8 to +inf, which the DVE clamp maps to e^5.

Engine plan: ScalarE does ONLY the exp (the 261us bottleneck; 256 x
[128,1024] PSUM->SBUF fp8). PE does the GEMMs (~282us). DVE does the
fp8 clamp (one [128,2048] 2x_2p op per pair), all PSUM evacuations, and
the normalization reciprocal. GPSIMD only preps (ones init, weight DMA).

Schedule (the whole point): one unified 8-pass loop (pass = head-pair x
n-half) paced by the activation engine, with all projection work injected
into per-chunk slots so no pass is PE-starved:
 - startup: madd DMA first on the scalar queue, then K0/V block 0 and the
   first Q group only -> first exp at ~11us (was 28us);
 - P@V pairs go through a depth-8 FIFO, so each P@V fires ~8 pairs after
   its exp: pass-boundary norm latency and V-projection JIT never stall
   the in-order PE queue;
 - V projection chunks 4-17 ride pass 0, 18-31 ride pass 1 just ahead of
   their FIFO-deferred consumers; K1/K2/K3 projections ride passes 1/2/3;
   Q groups ride startup..pass 3; out-projection groups (nh0) ride passes
   5-7; the pass-k normalization runs in pass k+1 split as reciprocal
   (mc15) + broadcast-multiply (mc17) so the PE never waits on the DVE.
"""
import sys
sys.path.insert(0, '/opt/trn_rl_repo')
from contextlib import ExitStack

import numpy as np
import ml_dtypes

import concourse.bass as bass  # noqa: F401
import concourse.mybir as mybir
import concourse.tile as tile
from concourse import bacc
from concourse.bass_utils import run_bass_kernel_spmd

F32 = mybir.dt.float32
F32R = mybir.dt.float32r
BF16 = mybir.dt.bfloat16
F8 = mybir.dt.float8e4
AF = mybir.ActivationFunctionType
ALU = mybir.AluOpType
DR = mybir.MatmulPerfMode.DoubleRow

B, N, M = 4, 1024, 4096
DQ = 1024
NHC = 8              # heads per core
D = 64
IC = NHC * D         # 512 inner dims per core
NP = NHC // 2        # 4 head pairs per core
MC = M // 128        # 32 context chunks of 128
CP = MC // 2         # 16 chunk pairs
VW = 96              # V pair-slot width: 64 dims + 32 ones columns
E5 = float(np.exp(np.float32(5.0)))
EM5 = float(np.exp(np.float32(-5.0)))
SCALE = float(D) ** -0.5  # 0.125
FIFO_D = 7           # P@V pairs deferred behind this many younger pairs

_CACHE = {}


def _emit(nc, tc, tensors, pfx=""):
    xt_d, ctxt_d, wq_d, wk_d, wv_d, wo_d, madd_d, out_d = tensors

    with nc.allow_low_precision(reason="bf16/fp8 matmul operands"), \
            ExitStack() as ctx:
        persist = ctx.enter_context(tc.tile_pool(name=f"{pfx}persist", bufs=1))
        pcl = ctx.enter_context(tc.tile_pool(name=f"{pfx}pcl", bufs=4))

        # -- input DMAs; queue order is load-bearing. scalar queue: madd
        # first (first exp needs it), then the two x^T halves. sync queue:
        # ctx^T blocks. gpsimd queue: weights.
        madd_sb = persist.tile([128, MC], F32, tag="madd")
        nc.scalar.dma_start(madd_sb[:], madd_d[:])

        def stage_ctx(m5, nm):
            t = pcl.tile([128, 8, 512], BF16, tag="ctxs", name=f"{pfx}cs{nm}")
            nc.sync.dma_start(
                t[:], ctxt_d[:, m5 * 512:(m5 + 1) * 512].rearrange(
                    "(c p) m -> p c m", p=128))
            return t

        # DMA priority: the first S chunk needs ctx0 + wk + xt-h0 + wq; wv
        # is needed by the first V chunk shortly after; everything else
        # (xt-h1, later ctx blocks) trails. ctx prefetch beyond block 0 is
        # deferred into pass-0 slots so the round-robin DMA service can't
        # starve the weights.
        ctx_tiles = {0: stage_ctx(0, "0")}
        wk_r = persist.tile([128, 8, IC], BF16, tag="wk")
        nc.gpsimd.dma_start(wk_r[:], wk_d.rearrange("(c p) i -> p c i", p=128))
        xt_s = persist.tile([128, 8, N], BF16, tag="xt")
        nc.scalar.dma_start(
            xt_s[:, :, 0:512],
            xt_d[:, 0:512].rearrange("(c p) n -> p c n", p=128))
        wq_r = persist.tile([128, 8, IC], BF16, tag="wq")
        nc.gpsimd.dma_start(wq_r[:], wq_d.rearrange("(c p) i -> p c i", p=128))
        wv_r = persist.tile([128, 8, IC], BF16, tag="wv")
        nc.gpsimd.dma_start(wv_r[:], wv_d.rearrange("(c p) i -> p c i", p=128))
        wo_r = persist.tile([128, NP, DQ], F32R, tag="wo")

        def load_xt1():
            nc.scalar.dma_start(
                xt_s[:, :, 512:1024],
                xt_d[:, 512:1024].rearrange("(c p) n -> p c n", p=128))

        ones_f = persist.tile([128, 1], F32, tag="onesf")
        nc.vector.memset(ones_f[:], 1.0)
        ones_8 = persist.tile([128, 1], F8, tag="ones8")
        nc.vector.tensor_copy(ones_8[:], ones_f[:])
        ones_r = persist.tile([1, 64], F32R, tag="onesr")
        nc.vector.tensor_copy(ones_r[:], ones_f[0:1, 0:1].to_broadcast((1, 64)))

        QT = [persist.tile([128, N], BF16, tag=f"qt{p}", name=f"{pfx}qt{p}")
              for p in range(NP)]
        KT = [persist.tile([128, M], BF16, tag=f"kt{p}", name=f"{pfx}kt{p}")
              for p in range(NP)]
        V = [persist.tile([128, NHC * 2 * VW], F8, tag=f"v{cp}",
                          name=f"{pfx}v{cp}") for cp in range(CP)]
        OnT = [persist.tile([128, N], F32R, tag=f"ont{p}", name=f"{pfx}ont{p}")
               for p in range(NP)]

        # ones columns of V never change: write them once up front
        for cp in range(CP):
            v4 = V[cp].rearrange("q (h two e) -> q h two e", two=2, e=VW)
            nc.vector.tensor_copy(
                v4[:, :, :, 64:VW],
                ones_8[:, 0:1, None, None].to_broadcast((128, NHC, 2, VW - 64)))

        pp = ctx.enter_context(tc.tile_pool(name=f"{pfx}pp", bufs=FIFO_D + 1))
        prb = ctx.enter_context(tc.tile_pool(name=f"{pfx}prb", bufs=2))
        pf = ctx.enter_context(tc.tile_pool(name=f"{pfx}pf", bufs=2))
        psS = ctx.enter_context(
            tc.tile_pool(name=f"{pfx}psS", bufs=2, space="PSUM"))
        psO = ctx.enter_context(
            tc.tile_pool(name=f"{pfx}psO", bufs=1, space="PSUM"))

        # ---------------- helpers ----------------
        def k_block(kp_, mb, ctx_s):
            kp = psS.tile([128, 512], F32, tag="kv", name=f"{pfx}k{kp_}_{mb}")
            for dc in range(8):
                nc.tensor.matmul(kp[:], wk_r[:, dc, kp_ * 128:(kp_ + 1) * 128],
                                 ctx_s[:, dc, :], start=(dc == 0), stop=(dc == 7))
            nc.vector.tensor_copy(KT[kp_][:, mb * 512:(mb + 1) * 512], kp[:])

        def v_chunk(mc, ctx_s):
            s = mc % 4
            vp = psS.tile([128, 512], F32, tag="kv", name=f"{pfx}v{mc}")
            for dc in range(8):
                nc.tensor.matmul(vp[:], ctx_s[:, dc, s * 128:(s + 1) * 128],
                                 wv_r[:, dc, :], start=(dc == 0), stop=(dc == 7))
            v4 = V[mc // 2].rearrange("q (h two e) -> q h two e", two=2, e=VW)
            nc.vector.tensor_copy(
                v4[:, :, mc % 2:mc % 2 + 1, 0:64],
                vp[:].rearrange("q (h e) -> q h e", e=64)[:, :, None, :])

        def q_group(nh, p):
            qp = psS.tile([128, 512], F32, tag="kv", name=f"{pfx}q{nh}_{p}")
            for dc in range(8):
                nc.tensor.matmul(qp[:], wq_r[:, dc, p * 128:(p + 1) * 128],
                                 xt_s[:, dc, nh * 512:(nh + 1) * 512],
                                 start=(dc == 0), stop=(dc == 7))
            nc.vector.tensor_copy(QT[p][:, nh * 512:(nh + 1) * 512], qp[:])

        def att_S(p, nh, mc, P2):
            S = psS.tile([128, 1024], F32, tag="s", name=f"{pfx}s{p}_{nh}_{mc}")
            nc.tensor.matmul(S[:, 0:512], KT[p][0:64, mc * 128:(mc + 1) * 128],
                             QT[p][0:64, nh * 512:(nh + 1) * 512],
                             start=True, stop=True, tile_position=(0, 0))
            nc.tensor.matmul(S[:, 512:1024],
                             KT[p][64:128, mc * 128:(mc + 1) * 128],
                             QT[p][64:128, nh * 512:(nh + 1) * 512],
                             start=True, stop=True, tile_position=(64, 0))
            j = mc % 2
            nc.scalar.activation(P2[:, j * 1024:(j + 1) * 1024], S[:], AF.Exp,
                                 bias=madd_sb[:, mc:mc + 1], scale=SCALE)

        def att_PV(p, cp, P2, O):
            P3 = P2.rearrange("q (two n) -> q two n", two=2)
            v4 = V[cp].rearrange("q (h two e) -> q h two e", two=2, e=VW)
            for h2 in range(2):
                nc.tensor.matmul(O[h2][:], v4[:, 2 * p + h2],
                                 P3[:, :, h2 * 512:(h2 + 1) * 512],
                                 start=(cp == 0), stop=(cp == CP - 1),
                                 perf_mode=DR)

        # nh1 output groups: 3 of the 4 OnT pairs are pre-accumulated into
        # SBUF during pass 7; the drain only adds the last pair.
        obp = [persist.tile([128, 512], F32, tag=f"obp{g}",
                            name=f"{pfx}obp{g}") for g in range(5)]

        def partial_out(g):
            n8, dqh = 4 + g // 2, g % 2
            po = psS.tile([128, 512], F32, tag="kv", name=f"{pfx}pp{g}")
            for p2 in range(3):
                nc.tensor.matmul(
                    po[:], OnT[p2][:, n8 * 128:(n8 + 1) * 128],
                    wo_r[:, p2, dqh * 512:(dqh + 1) * 512],
                    start=(p2 == 0), stop=(p2 == 2))
            nc.vector.tensor_copy(obp[g][:], po[:])

        def final_out(g):
            # drain path: alternate the dead "s" ring with "kv" for 4-deep
            # pipelining; add in-place into obp (it is dead after this) so
            # no pf ring bottleneck either.
            n8, dqh = 4 + g // 2, g % 2
            po = psS.tile([128, 512], F32, tag=("s" if g % 2 else "kv"),
                          name=f"{pfx}pq{g}")
            if g < 5:
                nc.tensor.matmul(po[:], OnT[3][:, n8 * 128:(n8 + 1) * 128],
                                 wo_r[:, 3, dqh * 512:(dqh + 1) * 512],
                                 start=True, stop=True)
                nc.vector.tensor_tensor(obp[g][:], obp[g][:], po[:], ALU.add)
                ob = obp[g]
            else:
                for p2 in range(NP):
                    nc.tensor.matmul(
                        po[:], OnT[p2][:, n8 * 128:(n8 + 1) * 128],
                        wo_r[:, p2, dqh * 512:(dqh + 1) * 512],
                        start=(p2 == 0), stop=(p2 == NP - 1))
                ob = pf.tile([128, 512], F32, tag="ob", name=f"{pfx}fo{g}")
                nc.vector.tensor_copy(ob[:], po[:])
            nc.sync.dma_start(
                out_d[n8 * 128:(n8 + 1) * 128, dqh * 512:(dqh + 1) * 512],
                ob[:])

        fifo = []

        def pv_push(entry, drain_old=False, drain_tail=0):
            fifo.append(entry)
            if len(fifo) > FIFO_D:
                att_PV(*fifo.pop(0)[1:])
            if drain_old:
                # boundary fast-drain: flush the previous pass's leftovers
                # early (capped per slot) so its normalization - and O-bank
                # reuse - never stalls the in-order PE queue
                for _ in range(2):
                    if fifo and fifo[0][0] != entry[0] and len(fifo) > 2:
                        att_PV(*fifo.pop(0)[1:])
            for _ in range(drain_tail):
                if fifo:
                    att_PV(*fifo.pop(0)[1:])

        def norm_recip(O2, p, nh):
            rcs = []
            for h2 in range(2):
                rc = prb.tile([1, 512], F32R, tag="rc",
                              name=f"{pfx}rc{p}{nh}{h2}")
                nc.vector.reciprocal(rc[:], O2[h2][64:65, :])
                rcs.append(rc)
            return rcs

        def norm_apply(rcs, O2, p, nh):
            # Rb rides the "kv" ring, NOT "s": recycling the hot S ring here
            # stalls S production (and the activation engine) at every pass
            # boundary while the DVE drains the norm reads.
            for h2 in range(2):
                Rb = psS.tile([64, 512], F32, tag="kv",
                              name=f"{pfx}rb{p}{nh}{h2}")
                nc.tensor.matmul(Rb[:], ones_r[:], rcs[h2][:],
                                 start=True, stop=True)
                rbs = prb.tile([64, 512], F32, tag="rbs",
                               name=f"{pfx}rbs{p}{nh}{h2}")
                nc.vector.tensor_copy(rbs[:], Rb[:])
                nc.vector.tensor_tensor(
                    OnT[p][h2 * 64:(h2 + 1) * 64, nh * 512:(nh + 1) * 512],
                    O2[h2][0:64, :], rbs[:], ALU.mult)

        def out_group(n8, dqh):
            po = psS.tile([128, 512], F32, tag="kv", name=f"{pfx}po{n8}_{dqh}")
            for p2 in range(NP):
                nc.tensor.matmul(
                    po[:], OnT[p2][:, n8 * 128:(n8 + 1) * 128],
                    wo_r[:, p2, dqh * 512:(dqh + 1) * 512],
                    start=(p2 == 0), stop=(p2 == NP - 1))
            ob = pf.tile([128, 512], F32, tag="ob", name=f"{pfx}ob{n8}_{dqh}")
            nc.vector.tensor_copy(ob[:], po[:])
            nc.sync.dma_start(
                out_d[n8 * 128:(n8 + 1) * 128, dqh * 512:(dqh + 1) * 512],
                ob[:])

        # ---------------- startup (A0) ----------------
        # Just K0 block 0 + the first Q group: S(0,0,0) - and the first
        # exp - start as soon as ctx0/wk/xt-h0/wq land (~13us). V chunks
        # ride pass-0 slots (after slot 4, so the first S chunks never
        # wait on the later wv DMA).
        k_block(0, 0, ctx_tiles[0])
        q_group(0, 0)
        ctx_tiles[1] = stage_ctx(1, "1")

        # ---------------- injection schedule ----------------
        # inj[pi][mc] -> list of thunks run right after att_S of that chunk.
        inj = [dict() for _ in range(8)]

        def add(pi, mc, fn):
            inj[pi].setdefault(mc, []).append(fn)

        # pass 0: K0 blocks 1-7 JIT, V chunks 0-17, Q group (1,0); ctx
        # staging spread so its DMAs trail the weight DMAs, and always
        # emitted after the same-slot V chunk that reads the evicted tile.
        v_slots0 = [5, 6, 7, 9, 10, 11, 13, 14, 15, 17, 18, 19, 21, 22, 23,
                    25, 26, 27]
        for i, sl in enumerate(v_slots0):
            add(0, sl, lambda mc=i: v_chunk(mc, ctx_tiles[mc // 4]))
        add(0, 1, lambda: ctx_tiles.__setitem__(2, stage_ctx(2, "2")))
        add(0, 4, load_xt1)
        for sl, b in ((7, 3), (11, 4), (15, 5), (19, 6), (25, 7)):
            add(0, sl, lambda b=b: ctx_tiles.__setitem__(
                b, stage_ctx(b, str(b))))
        for b in range(1, 8):
            add(0, 4 * b, lambda b=b: k_block(0, b, ctx_tiles[b]))
        add(0, 29, lambda: q_group(1, 0))

        # pass 1: V chunks 18-31 just ahead of the FIFO-drained pass-0 P@Vs,
        # K1 blocks on the late slots, Q groups (0,1) and (0,2).
        v_slots1 = [0, 0, 1, 2, 3, 4, 5, 6, 7, 8, 9, 10, 11, 12]
        for i, sl in enumerate(v_slots1):
            add(1, sl, lambda mc=18 + i: v_chunk(mc, ctx_tiles[mc // 4]))
        kctx = {}
        for b in range(8):
            add(1, 12 + 2 * b, lambda b=b, kp_=1: kctx.__setitem__(
                (kp_, b), stage_ctx(b, f"k{kp_}_{b}")))
            add(1, 16 + 2 * b, lambda b=b, kp_=1: k_block(
                kp_, b, kctx.pop((kp_, b))))
        add(1, 21, lambda: q_group(0, 1))
        add(1, 23, lambda: q_group(0, 2))

        # passes 2/3: K2/K3 JIT (ctx staged 2 blocks ahead; first two blocks
        # staged at the tail of the previous pass), remaining Q groups.
        for pi, kp_ in ((2, 2), (3, 3)):
            for b in range(2):
                add(pi - 1, 28 + 2 * b, lambda b=b, kp_=kp_: kctx.__setitem__(
                    (kp_, b), stage_ctx(b, f"k{kp_}_{b}")))
            for b in range(8):
                if b + 2 < 8:
                    add(pi, 4 * b, lambda b=b, kp_=kp_: kctx.__setitem__(
                        (kp_, b + 2), stage_ctx(b + 2, f"k{kp_}_{b + 2}")))
                add(pi, 4 * b, lambda b=b, kp_=kp_: k_block(
                    kp_, b, kctx.pop((kp_, b))))
        add(2, 0, lambda: nc.gpsimd.dma_start(
            wo_r[:], wo_d.rearrange("(p q) d -> q p d", q=128)))
        add(2, 17, lambda: q_group(1, 1))
        add(2, 19, lambda: q_group(1, 2))
        add(2, 21, lambda: q_group(0, 3))
        add(3, 17, lambda: q_group(1, 3))

        # passes 5-7: output projection for the nh0 half (n8 0-3)
        og = [(n8, dqh) for n8 in range(4) for dqh in range(2)]
        for pi, sl in ((5, 21), (5, 25), (5, 29), (6, 5), (6, 13), (6, 21),
                       (6, 29), (7, 5)):
            add(pi, sl, lambda g=og.pop(0): out_group(*g))
        for g in range(5):
            add(7, 12 + 2 * g, lambda g=g: partial_out(g))

        # ---------------- unified pass loop ----------------
        passes = [(0, 0), (0, 1), (1, 0), (2, 0), (3, 0), (1, 1), (2, 1),
                  (3, 1)]
        norm_state = None      # (O_tiles, p, nh) of the previous pass
        rcs_state = None
        for pi, (p, nh) in enumerate(passes):
            O_cur = [psO.tile([VW, 512], F32, tag=f"oo{h2}",
                              name=f"{pfx}o{p}_{nh}_{h2}") for h2 in range(2)]
            P2 = None
            for mc in range(MC):
                if mc % 2 == 0:
                    P2 = pp.tile([128, 2048], F8, tag="p",
                                 name=f"{pfx}p{p}_{nh}_{mc // 2}")
                # injections first: same-slot K blocks must be emitted
                # before the S chunk that reads their KT columns
                for fn in inj[pi].get(mc, []):
                    fn()
                att_S(p, nh, mc, P2)
                apply_sl = 11 if pi >= 2 else 17
                if mc == apply_sl and rcs_state is not None:
                    norm_apply(rcs_state, *norm_state)
                    norm_state = rcs_state = None
                if mc % 2 == 1:
                    nc.vector.tensor_scalar(P2[:], P2[:], E5, EM5,
                                            ALU.min, ALU.max)
                    pv_push((pi, p, mc // 2, P2, O_cur),
                            drain_old=(pi >= 2 and mc <= 7),
                            drain_tail=(2 if pi == 7 and mc >= 17 else 0))
                    if mc == apply_sl - 2 and norm_state is not None:
                        rcs_state = norm_recip(*norm_state)
            norm_state = (O_cur, p, nh)

        # ---------------- drain ----------------
        while fifo:
            att_PV(*fifo.pop(0)[1:])
        rcs = norm_recip(*norm_state)
        norm_apply(rcs, *norm_state)
        for g in range(8):
            final_out(g)


def _build(n_bodies=1):
    nc = bacc.Bacc("TRN2", target_bir_lowering=False, debug=False, num_devices=8)
    xt_d = nc.dram_tensor("xt", [DQ, N], BF16, kind="ExternalInput")
    ctxt_d = nc.dram_tensor("ctxt", [DQ, M], BF16, kind="ExternalInput")
    wq_d = nc.dram_tensor("wq", [DQ, IC], BF16, kind="ExternalInput")
    wk_d = nc.dram_tensor("wk", [DQ, IC], BF16, kind="ExternalInput")
    wv_d = nc.dram_tensor("wv", [DQ, IC], BF16, kind="ExternalInput")
    wo_d = nc.dram_tensor("wo", [IC, DQ], F32, kind="ExternalInput")
    madd_d = nc.dram_tensor("madd", [128, MC], F32, kind="ExternalInput")
    out_d = nc.dram_tensor("out", [N, DQ], F32, kind="ExternalOutput")
    with tile.TileContext(nc) as tc:
        for i in range(n_bodies):
            _emit(nc, tc, (xt_d, ctxt_d, wq_d, wk_d, wv_d, wo_d, madd_d, out_d),
                  pfx=(f"b{i}_" if n_bodies > 1 else ""))
    nc.compile()
    return nc


def _in_maps(x, context, mask, Wq, Wkv, Wo):
    bf = ml_dtypes.bfloat16
    maps = []
    for c in range(8):
        b, hh = divmod(c, 2)
        cs = hh * IC
        madd = np.where(mask[b], np.float32(0.0), np.float32(-1000.0))
        madd = madd.astype(np.float32).reshape(MC, 128).T
        maps.append({
            "xt": np.ascontiguousarray(x[b].T.astype(bf)),
            "ctxt": np.ascontiguousarray(context[b].T.astype(bf)),
            "wq": np.ascontiguousarray(Wq[:, cs:cs + IC].astype(bf)),
            "wk": np.ascontiguousarray(Wkv[:, cs:cs + IC].astype(bf)),
            "wv": np.ascontiguousarray(Wkv[:, DQ + cs:DQ + cs + IC].astype(bf)),
            "wo": np.ascontiguousarray(Wo[cs:cs + IC, :]),
            "madd": np.ascontiguousarray(madd),
        })
    return maps


def kernel(x, context, mask, Wq, Wkv, Wo, bo):
    x = np.asarray(x, dtype=np.float32)
    context = np.asarray(context, dtype=np.float32)
    mask = np.asarray(mask)
    Wq = np.asarray(Wq, dtype=np.float32)
    Wkv = np.asarray(Wkv, dtype=np.float32)
    Wo = np.asarray(Wo, dtype=np.float32)
    bo = np.asarray(bo, dtype=np.float32)

    if "nc" not in _CACHE:
        _CACHE["nc"] = _build()
    nc = _CACHE["nc"]

    res = run_bass_kernel_spmd(nc, _in_maps(x, context, mask, Wq, Wkv, Wo),
                               core_ids=list(range(8)))
    _CACHE["last_results"] = res

    out = np.empty((B, N, DQ), dtype=np.float32)
    for b in range(B):
        out[b] = res.results[2 * b]["out"] + res.results[2 * b + 1]["out"] \
            + bo[None, :]
    return out


# revision 24
# speedup vs baseline: 1.0932x; 1.0932x over previous
"""Trainium2 Bass kernel for nn_CrossAttentionLayer (B=4, N=1024, M=4096,
DQ=DC=1024, H=16, DH=64).

Sharding: 8 cores = 4 batches x 2 half-head-groups. Core c handles batch
c//2 and heads [8*(c%2), 8*(c%2)+8). Each core computes its partial
out = concat_heads(attn) @ Wo_slice; host sums the two partials per batch
and adds the bias.

Math identity on device:
  P = clamp(exp(scale*S + madd), e^-5, e^5)   with madd = 0 / -1000 (mask)
equals exp(clip(where(mask, scale*S, -inf), -5, 5)).  x^T/ctx^T/Wq/Wkv are
cast + transposed on the host so the kernel does NO PE transposes; K^T,
Q^T and V stay resident in SBUF.

Dtypes: projections and S in bf16 (fp8 there fails the 2e-2 tolerance);
P@V runs in fp8-e4m3 DoubleRow (end-to-end max-rel-err 1.8e-2 emulated):
V is stored fp8 in chunk-pair layout [128, h(8), 2, 96] (64 V dims + 32
ones columns - DoubleRow stationary width must be a multiple of 32; P@V
output rows 64..95 accumulate sum(P), the softmax denominator) and the
exp writes P fp8 into pair tiles [128, 2*1024]; one DoubleRow matmul per
(chunk-pair, head) contracts both 128-row chunks at 0.5 cycles/column.
exp overflow saturates fp8 to +inf, which the DVE clamp maps to e^5.

Engine plan: ScalarE does ONLY the exp (the 261us bottleneck; 256 x
[128,1024] PSUM->SBUF fp8). PE does the GEMMs (~282us). DVE does the
fp8 clamp (one [128,2048] 2x_2p op per pair), all PSUM evacuations, and
the normalization reciprocal. GPSIMD only preps (ones init, weight DMA).

Schedule (the whole point): one unified 8-pass loop (pass = head-pair x
n-half) paced by the activation engine, with all projection work injected
into per-chunk slots so no pass is PE-starved:
 - startup: madd DMA first on the scalar queue, then K0/V block 0 and the
   first Q group only -> first exp at ~11us (was 28us);
 - P@V pairs go through a depth-8 FIFO, so each P@V fires ~8 pairs after
   its exp: pass-boundary norm latency and V-projection JIT never stall
   the in-order PE queue;
 - V projection chunks 4-17 ride pass 0, 18-31 ride pass 1 just ahead of
   their FIFO-deferred consumers; K1/K2/K3 projections ride passes 1/2/3;
   Q groups ride startup..pass 3; out-projection groups (nh0) ride passes
   5-7; the pass-k normalization runs in pass k+1 split as reciprocal
   (mc15) + broadcast-multiply (mc17) so the PE never waits on the DVE.
"""
import sys
sys.path.insert(0, '/opt/trn_rl_repo')
from contextlib import ExitStack

import numpy as np
import ml_dtypes

import concourse.bass as bass  # noqa: F401
import concourse.mybir as mybir
import concourse.tile as tile
from concourse import bacc
from concourse.bass_utils import run_bass_kernel_spmd

F32 = mybir.dt.float32
F32R = mybir.dt.float32r
BF16 = mybir.dt.bfloat16
F8 = mybir.dt.float8e4
AF = mybir.ActivationFunctionType
ALU = mybir.AluOpType
DR = mybir.MatmulPerfMode.DoubleRow

B, N, M = 4, 1024, 4096
DQ = 1024
NHC = 8              # heads per core
D = 64
IC = NHC * D         # 512 inner dims per core
NP = NHC // 2        # 4 head pairs per core
MC = M // 128        # 32 context chunks of 128
CP = MC // 2         # 16 chunk pairs
VW = 96              # V pair-slot width: 64 dims + 32 ones columns
E5 = float(np.exp(np.float32(5.0)))
EM5 = float(np.exp(np.float32(-5.0)))
SCALE = float(D) ** -0.5  # 0.125
FIFO_D = 7           # P@V pairs deferred behind this many younger pairs

_CACHE = {}


def _emit(nc, tc, tensors, pfx=""):
    xt_d, ctxt_d, wq_d, wk_d, wv_d, wo_d, madd_d, out_d = tensors

    with nc.allow_low_precision(reason="bf16/fp8 matmul operands"), \
            ExitStack() as ctx:
        persist = ctx.enter_context(tc.tile_pool(name=f"{pfx}persist", bufs=1))
        pcl = ctx.enter_context(tc.tile_pool(name=f"{pfx}pcl", bufs=4))

        # -- input DMAs; queue order is load-bearing. scalar queue: madd
        # first (first exp needs it), then the two x^T halves. sync queue:
        # ctx^T blocks. gpsimd queue: weights.
        madd_sb = persist.tile([128, MC], F32, tag="madd")
        nc.scalar.dma_start(madd_sb[:], madd_d[:])

        def stage_ctx(m5, nm):
            t = pcl.tile([128, 8, 512], BF16, tag="ctxs", name=f"{pfx}cs{nm}")
            nc.sync.dma_start(t[:], ctxt_d[m5])
            return t

        # DMA priority: the first S chunk needs ctx0 + the pair-0 slices of
        # wk/wq + xt-h0; wv follows for the first V chunk; the remaining
        # wk/wq columns, xt-h1 and later ctx blocks trail (injected). All
        # DRAM layouts are host-pre-arranged so every transfer is
        # partition-contiguous (one descriptor per partition).
        ctx_tiles = {0: stage_ctx(0, "0")}
        wk_r = persist.tile([128, 8, IC], BF16, tag="wk")
        nc.gpsimd.dma_start(wk_r[:, :, 0:128], wk_d[:, :, 0:128])
        xt_s = persist.tile([128, 2, 8, 512], BF16, tag="xt")
        nc.scalar.dma_start(xt_s[:, 0], xt_d[0])
        wq_r = persist.tile([128, 8, IC], BF16, tag="wq")
        nc.gpsimd.dma_start(wq_r[:, :, 0:128], wq_d[:, :, 0:128])
        wv_r = persist.tile([128, 8, IC], BF16, tag="wv")
        nc.gpsimd.dma_start(wv_r[:], wv_d[:])
        wo_r = persist.tile([128, NP, DQ], F32R, tag="wo")

        def load_xt1():
            nc.scalar.dma_start(xt_s[:, 1], xt_d[1])

        def load_w_rest():
            nc.gpsimd.dma_start(wk_r[:, :, 128:512], wk_d[:, :, 128:512])
            nc.gpsimd.dma_start(wq_r[:, :, 128:512], wq_d[:, :, 128:512])

        ones_f = persist.tile([128, 1], F32, tag="onesf")
        nc.vector.memset(ones_f[:], 1.0)
        ones_8 = persist.tile([128, 1], F8, tag="ones8")
        nc.vector.tensor_copy(ones_8[:], ones_f[:])
        ones_r = persist.tile([1, 64], F32R, tag="onesr")
        nc.vector.tensor_copy(ones_r[:], ones_f[0:1, 0:1].to_broadcast((1, 64)))

        QT = [persist.tile([128, N], BF16, tag=f"qt{p}", name=f"{pfx}qt{p}")
              for p in range(NP)]
        KT = [persist.tile([128, M], BF16, tag=f"kt{p}", name=f"{pfx}kt{p}")
              for p in range(NP)]
        V = [persist.tile([128, NHC * 2 * VW], F8, tag=f"v{cp}",
                          name=f"{pfx}v{cp}") for cp in range(CP)]
        OnT = [persist.tile([128, N], F32R, tag=f"ont{p}", name=f"{pfx}ont{p}")
               for p in range(NP)]

        # ones columns of V never change: write them once up front
        for cp in range(CP):
            v4 = V[cp].rearrange("q (h two e) -> q h two e", two=2, e=VW)
            nc.vector.tensor_copy(
                v4[:, :, :, 64:VW],
                ones_8[:, 0:1, None, None].to_broadcast((128, NHC, 2, VW - 64)))

        pp = ctx.enter_context(tc.tile_pool(name=f"{pfx}pp", bufs=FIFO_D + 1))
        prb = ctx.enter_context(tc.tile_pool(name=f"{pfx}prb", bufs=2))
        pf = ctx.enter_context(tc.tile_pool(name=f"{pfx}pf", bufs=2))
        psS = ctx.enter_context(
            tc.tile_pool(name=f"{pfx}psS", bufs=2, space="PSUM"))
        psO = ctx.enter_context(
            tc.tile_pool(name=f"{pfx}psO", bufs=1, space="PSUM"))

        # ---------------- helpers ----------------
        def k_block(kp_, mb, ctx_s):
            kp = psS.tile([128, 512], F32, tag="kv", name=f"{pfx}k{kp_}_{mb}")
            for dc in range(8):
                nc.tensor.matmul(kp[:], wk_r[:, dc, kp_ * 128:(kp_ + 1) * 128],
                                 ctx_s[:, dc, :], start=(dc == 0), stop=(dc == 7))
            nc.vector.tensor_copy(KT[kp_][:, mb * 512:(mb + 1) * 512], kp[:])

        def v_chunk(mc, ctx_s):
            s = mc % 4
            vp = psS.tile([128, 512], F32, tag="kv", name=f"{pfx}v{mc}")
            for dc in range(8):
                nc.tensor.matmul(vp[:], ctx_s[:, dc, s * 128:(s + 1) * 128],
                                 wv_r[:, dc, :], start=(dc == 0), stop=(dc == 7))
            v4 = V[mc // 2].rearrange("q (h two e) -> q h two e", two=2, e=VW)
            nc.vector.tensor_copy(
                v4[:, :, mc % 2:mc % 2 + 1, 0:64],
                vp[:].rearrange("q (h e) -> q h e", e=64)[:, :, None, :])

        def q_group(nh, p):
            qp = psS.tile([128, 512], F32, tag="kv", name=f"{pfx}q{nh}_{p}")
            for dc in range(8):
                nc.tensor.matmul(qp[:], wq_r[:, dc, p * 128:(p + 1) * 128],
                                 xt_s[:, nh, dc, :],
                                 start=(dc == 0), stop=(dc == 7))
            nc.vector.tensor_copy(QT[p][:, nh * 512:(nh + 1) * 512], qp[:])

        def att_S(p, nh, mc, P2):
            S = psS.tile([128, 1024], F32, tag="s", name=f"{pfx}s{p}_{nh}_{mc}")
            nc.tensor.matmul(S[:, 0:512], KT[p][0:64, mc * 128:(mc + 1) * 128],
                             QT[p][0:64, nh * 512:(nh + 1) * 512],
                             start=True, stop=True, tile_position=(0, 0))
            nc.tensor.matmul(S[:, 512:1024],
                             KT[p][64:128, mc * 128:(mc + 1) * 128],
                             QT[p][64:128, nh * 512:(nh + 1) * 512],
                             start=True, stop=True, tile_position=(64, 0))
            j = mc % 2
            nc.scalar.activation(P2[:, j * 1024:(j + 1) * 1024], S[:], AF.Exp,
                                 bias=madd_sb[:, mc:mc + 1], scale=SCALE)

        def att_PV(p, cp, P2, O):
            P3 = P2.rearrange("q (two n) -> q two n", two=2)
            v4 = V[cp].rearrange("q (h two e) -> q h two e", two=2, e=VW)
            for h2 in range(2):
                nc.tensor.matmul(O[h2][:], v4[:, 2 * p + h2],
                                 P3[:, :, h2 * 512:(h2 + 1) * 512],
                                 start=(cp == 0), stop=(cp == CP - 1),
                                 perf_mode=DR)

        # nh1 output groups: 3 of the 4 OnT pairs are pre-accumulated into
        # SBUF during pass 7; the drain only adds the last pair.
        obp = [persist.tile([128, 512], F32, tag=f"obp{g}",
                            name=f"{pfx}obp{g}") for g in range(5)]

        def partial_out(g):
            n8, dqh = 4 + g // 2, g % 2
            po = psS.tile([128, 512], F32, tag="kv", name=f"{pfx}pp{g}")
            for p2 in range(3):
                nc.tensor.matmul(
                    po[:], OnT[p2][:, n8 * 128:(n8 + 1) * 128],
                    wo_r[:, p2, dqh * 512:(dqh + 1) * 512],
                    start=(p2 == 0), stop=(p2 == 2))
            nc.vector.tensor_copy(obp[g][:], po[:])

        def final_out(g):
            # drain path: alternate the dead "s" ring with "kv" for 4-deep
            # pipelining; add in-place into obp (it is dead after this) so
            # no pf ring bottleneck either.
            n8, dqh = 4 + g // 2, g % 2
            po = psS.tile([128, 512], F32, tag=("s" if g % 2 else "kv"),
                          name=f"{pfx}pq{g}")
            if g < 5:
                nc.tensor.matmul(po[:], OnT[3][:, n8 * 128:(n8 + 1) * 128],
                                 wo_r[:, 3, dqh * 512:(dqh + 1) * 512],
                                 start=True, stop=True)
                nc.vector.tensor_tensor(obp[g][:], obp[g][:], po[:], ALU.add)
                ob = obp[g]
            else:
                for p2 in range(NP):
                    nc.tensor.matmul(
                        po[:], OnT[p2][:, n8 * 128:(n8 + 1) * 128],
                        wo_r[:, p2, dqh * 512:(dqh + 1) * 512],
                        start=(p2 == 0), stop=(p2 == NP - 1))
                ob = pf.tile([128, 512], F32, tag="ob", name=f"{pfx}fo{g}")
                nc.vector.tensor_copy(ob[:], po[:])
            nc.sync.dma_start(
                out_d[n8 * 128:(n8 + 1) * 128, dqh * 512:(dqh + 1) * 512],
                ob[:])

        fifo = []

        def pv_push(entry, drain_old=False, drain_tail=0):
            fifo.append(entry)
            if len(fifo) > FIFO_D:
                att_PV(*fifo.pop(0)[1:])
            if drain_old:
                # boundary fast-drain: flush the previous pass's leftovers
                # early (capped per slot) so its normalization - and O-bank
                # reuse - never stalls the in-order PE queue
                for _ in range(2):
                    if fifo and fifo[0][0] != entry[0] and len(fifo) > 2:
                        att_PV(*fifo.pop(0)[1:])
            for _ in range(drain_tail):
                if fifo:
                    att_PV(*fifo.pop(0)[1:])

        def norm_recip(O2, p, nh):
            rcs = []
            for h2 in range(2):
                rc = prb.tile([1, 512], F32R, tag="rc",
                              name=f"{pfx}rc{p}{nh}{h2}")
                nc.vector.reciprocal(rc[:], O2[h2][64:65, :])
                rcs.append(rc)
            return rcs

        def norm_apply(rcs, O2, p, nh):
            # Rb rides the "kv" ring, NOT "s": recycling the hot S ring here
            # stalls S production (and the activation engine) at every pass
            # boundary while the DVE drains the norm reads.
            for h2 in range(2):
                Rb = psS.tile([64, 512], F32, tag="kv",
                              name=f"{pfx}rb{p}{nh}{h2}")
                nc.tensor.matmul(Rb[:], ones_r[:], rcs[h2][:],
                                 start=True, stop=True)
                rbs = prb.tile([64, 512], F32, tag="rbs",
                               name=f"{pfx}rbs{p}{nh}{h2}")
                nc.vector.tensor_copy(rbs[:], Rb[:])
                nc.vector.tensor_tensor(
                    OnT[p][h2 * 64:(h2 + 1) * 64, nh * 512:(nh + 1) * 512],
                    O2[h2][0:64, :], rbs[:], ALU.mult)

        def out_group(n8, dqh):
            po = psS.tile([128, 512], F32, tag="kv", name=f"{pfx}po{n8}_{dqh}")
            for p2 in range(NP):
                nc.tensor.matmul(
                    po[:], OnT[p2][:, n8 * 128:(n8 + 1) * 128],
                    wo_r[:, p2, dqh * 512:(dqh + 1) * 512],
                    start=(p2 == 0), stop=(p2 == NP - 1))
            ob = pf.tile([128, 512], F32, tag="ob", name=f"{pfx}ob{n8}_{dqh}")
            nc.vector.tensor_copy(ob[:], po[:])
            nc.sync.dma_start(
                out_d[n8 * 128:(n8 + 1) * 128, dqh * 512:(dqh + 1) * 512],
                ob[:])

        # ---------------- startup (A0) ----------------
        # Just K0 block 0 + the first Q group: S(0,0,0) - and the first
        # exp - start as soon as ctx0/wk/xt-h0/wq land (~13us). V chunks
        # ride pass-0 slots (after slot 4, so the first S chunks never
        # wait on the later wv DMA).
        k_block(0, 0, ctx_tiles[0])
        q_group(0, 0)
        ctx_tiles[1] = stage_ctx(1, "1")

        # ---------------- injection schedule ----------------
        # inj[pi][mc] -> list of thunks run right after att_S of that chunk.
        inj = [dict() for _ in range(8)]

        def add(pi, mc, fn):
            inj[pi].setdefault(mc, []).append(fn)

        # pass 0: K0 blocks 1-7 JIT, V chunks 0-17, Q group (1,0); ctx
        # staging spread so its DMAs trail the weight DMAs, and always
        # emitted after the same-slot V chunk that reads the evicted tile.
        v_slots0 = [5, 6, 7, 9, 10, 11, 13, 14, 15, 17, 18, 19, 21, 22, 23,
                    25, 26, 27]
        for i, sl in enumerate(v_slots0):
            add(0, sl, lambda mc=i: v_chunk(mc, ctx_tiles[mc // 4]))
        add(0, 1, lambda: ctx_tiles.__setitem__(2, stage_ctx(2, "2")))
        add(0, 4, load_xt1)
        add(0, 6, load_w_rest)
        for sl, b in ((7, 3), (11, 4), (15, 5), (19, 6), (25, 7)):
            add(0, sl, lambda b=b: ctx_tiles.__setitem__(
                b, stage_ctx(b, str(b))))
        for b in range(1, 8):
            add(0, 4 * b, lambda b=b: k_block(0, b, ctx_tiles[b]))
        add(0, 29, lambda: q_group(1, 0))

        # pass 1: V chunks 18-31 just ahead of the FIFO-drained pass-0 P@Vs,
        # K1 blocks on the late slots, Q groups (0,1) and (0,2).
        v_slots1 = [0, 0, 1, 2, 3, 4, 5, 6, 7, 8, 9, 10, 11, 12]
        for i, sl in enumerate(v_slots1):
            add(1, sl, lambda mc=18 + i: v_chunk(mc, ctx_tiles[mc // 4]))
        kctx = {}
        for b in range(8):
            add(1, 12 + 2 * b, lambda b=b, kp_=1: kctx.__setitem__(
                (kp_, b), stage_ctx(b, f"k{kp_}_{b}")))
            add(1, 16 + 2 * b, lambda b=b, kp_=1: k_block(
                kp_, b, kctx.pop((kp_, b))))
        add(1, 21, lambda: q_group(0, 1))
        add(1, 23, lambda: q_group(0, 2))

        # passes 2/3: K2/K3 JIT (ctx staged 2 blocks ahead; first two blocks
        # staged at the tail of the previous pass), remaining Q groups.
        for pi, kp_ in ((2, 2), (3, 3)):
            for b in range(2):
                add(pi - 1, 28 + 2 * b, lambda b=b, kp_=kp_: kctx.__setitem__(
                    (kp_, b), stage_ctx(b, f"k{kp_}_{b}")))
            for b in range(8):
                if b + 2 < 8:
                    add(pi, 4 * b, lambda b=b, kp_=kp_: kctx.__setitem__(
                        (kp_, b + 2), stage_ctx(b + 2, f"k{kp_}_{b + 2}")))
                add(pi, 4 * b, lambda b=b, kp_=kp_: k_block(
                    kp_, b, kctx.pop((kp_, b))))
        add(2, 0, lambda: nc.gpsimd.dma_start(wo_r[:], wo_d[:]))
        add(2, 17, lambda: q_group(1, 1))
        add(2, 19, lambda: q_group(1, 2))
        add(2, 21, lambda: q_group(0, 3))
        add(3, 17, lambda: q_group(1, 3))

        # passes 5-7: output projection for the nh0 half (n8 0-3)
        og = [(n8, dqh) for n8 in range(4) for dqh in range(2)]
        for pi, sl in ((5, 21), (5, 25), (5, 29), (6, 5), (6, 13), (6, 21),
                       (6, 29), (7, 5)):
            add(pi, sl, lambda g=og.pop(0): out_group(*g))
        for g in range(5):
            add(7, 12 + 2 * g, lambda g=g: partial_out(g))

        # ---------------- unified pass loop ----------------
        passes = [(0, 0), (0, 1), (1, 0), (2, 0), (3, 0), (1, 1), (2, 1),
                  (3, 1)]
        norm_state = None      # (O_tiles, p, nh) of the previous pass
        rcs_state = None
        for pi, (p, nh) in enumerate(passes):
            O_cur = [psO.tile([VW, 512], F32, tag=f"oo{h2}",
                              name=f"{pfx}o{p}_{nh}_{h2}") for h2 in range(2)]
            P2 = None
            for mc in range(MC):
                if mc % 2 == 0:
                    P2 = pp.tile([128, 2048], F8, tag="p",
                                 name=f"{pfx}p{p}_{nh}_{mc // 2}")
                # injections first: same-slot K blocks must be emitted
                # before the S chunk that reads their KT columns
                for fn in inj[pi].get(mc, []):
                    fn()
                att_S(p, nh, mc, P2)
                apply_sl = 11 if pi >= 2 else 17
                if mc == apply_sl and rcs_state is not None:
                    norm_apply(rcs_state, *norm_state)
                    norm_state = rcs_state = None
                if mc % 2 == 1:
                    eng = nc.gpsimd if mc % 8 == 3 else nc.vector
                    eng.tensor_scalar(P2[:], P2[:], E5, EM5,
                                      ALU.min, ALU.max)
                    pv_push((pi, p, mc // 2, P2, O_cur),
                            drain_old=(pi >= 2 and mc <= 7),
                            drain_tail=(2 if pi == 7 and mc >= 17 else 0))
                    if mc == apply_sl - 2 and norm_state is not None:
                        rcs_state = norm_recip(*norm_state)
            norm_state = (O_cur, p, nh)

        # ---------------- drain ----------------
        while fifo:
            att_PV(*fifo.pop(0)[1:])
        rcs = norm_recip(*norm_state)
        norm_apply(rcs, *norm_state)
        for g in range(8):
            final_out(g)


def _build(n_bodies=1):
    nc = bacc.Bacc("TRN2", target_bir_lowering=False, debug=False, num_devices=8)
    xt_d = nc.dram_tensor("xt", [2, 128, 8, 512], BF16, kind="ExternalInput")
    ctxt_d = nc.dram_tensor("ctxt", [8, 128, 8, 512], BF16,
                            kind="ExternalInput")
    wq_d = nc.dram_tensor("wq", [128, 8, IC], BF16, kind="ExternalInput")
    wk_d = nc.dram_tensor("wk", [128, 8, IC], BF16, kind="ExternalInput")
    wv_d = nc.dram_tensor("wv", [128, 8, IC], BF16, kind="ExternalInput")
    wo_d = nc.dram_tensor("wo", [128, NP, DQ], F32, kind="ExternalInput")
    madd_d = nc.dram_tensor("madd", [128, MC], F32, kind="ExternalInput")
    out_d = nc.dram_tensor("out", [N, DQ], F32, kind="ExternalOutput")
    with tile.TileContext(nc) as tc:
        for i in range(n_bodies):
            _emit(nc, tc, (xt_d, ctxt_d, wq_d, wk_d, wv_d, wo_d, madd_d, out_d),
                  pfx=(f"b{i}_" if n_bodies > 1 else ""))
    nc.compile()
    return nc


def _in_maps(x, context, mask, Wq, Wkv, Wo):
    bf = ml_dtypes.bfloat16

    def wlay(w):       # [DQ, IC] -> [128(p), 8(c), IC] contiguous
        return np.ascontiguousarray(
            w.reshape(8, 128, IC).transpose(1, 0, 2).astype(bf))

    maps = []
    for c in range(8):
        b, hh = divmod(c, 2)
        cs = hh * IC
        madd = np.where(mask[b], np.float32(0.0), np.float32(-1000.0))
        madd = madd.astype(np.float32).reshape(MC, 128).T
        xT = x[b].T.astype(bf)          # [DQ, N]
        ctxT = context[b].T.astype(bf)  # [DQ, M]
        maps.append({
            "xt": np.ascontiguousarray(
                xT.reshape(8, 128, 2, 512).transpose(2, 1, 0, 3)),
            "ctxt": np.ascontiguousarray(
                ctxT.reshape(8, 128, 8, 512).transpose(2, 1, 0, 3)),
            "wq": wlay(Wq[:, cs:cs + IC]),
            "wk": wlay(Wkv[:, cs:cs + IC]),
            "wv": wlay(Wkv[:, DQ + cs:DQ + cs + IC]),
            "wo": np.ascontiguousarray(
                Wo[cs:cs + IC, :].reshape(NP, 128, DQ).transpose(
                    1, 0, 2).astype(np.float32)),
            "madd": np.ascontiguousarray(madd),
        })
    return maps


def kernel(x, context, mask, Wq, Wkv, Wo, bo):
    x = np.asarray(x, dtype=np.float32)
    context = np.asarray(context, dtype=np.float32)
    mask = np.asarray(mask)
    Wq = np.asarray(Wq, dtype=np.float32)
    Wkv = np.asarray(Wkv, dtype=np.float32)
    Wo = np.asarray(Wo, dtype=np.float32)
    bo = np.asarray(bo, dtype=np.float32)

    if "nc" not in _CACHE:
        _CACHE["nc"] = _build()
    nc = _CACHE["nc"]

    res = run_bass_kernel_spmd(nc, _in_maps(x, context, mask, Wq, Wkv, Wo),
                               core_ids=list(range(8)))
    _CACHE["last_results"] = res

    out = np.empty((B, N, DQ), dtype=np.float32)
    for b in range(B):
        out[b] = res.results[2 * b]["out"] + res.results[2 * b + 1]["out"] \
            + bo[None, :]
    return out
